# revision 1
# baseline (speedup 1.0000x reference)
"""Trainium2 Bass kernel for nn_BasicAttention (B=8, C=1024, L=2048, A=128).

Sharding: data-parallel over batch B — one example per NeuronCore, no
collectives.

Math (per example), using associativity to avoid any on-device transpose:
    keys    = Wk @ x + bk                      [A, L]
    queries = Wq @ x + bq                      [A, L]
    V       = keys^T @ queries                 [L, L]
    E       = exp(V / (L/2))   (raw exp; logits are ~1e-2 so no max-sub)
    S[l]    = sum_m E[l, m]
    yT      = x^T @ Wp^T       (= (Wp @ x)^T)  [L, C]
    out     = (yT / S)^T @ E + bp              [C, L]

The PE convention matmul(out, lhsT, rhs) = lhsT.T @ rhs with the
contraction on the partition dim lets every GEMM run without transposing
activations: host passes Wk^T/Wq^T/Wp^T packed into per-partition blobs,
x tiles serve directly as lhsT for yT, keys serve directly as lhsT for
V, and yT serves directly as lhsT for the final GEMM. E is staged
through DRAM between the values phase (row-major over l) and the final
phase (column-chunk-major over m).

This execution environment pays a large per-unique-instruction fetch
cost, so the kernel is structured as four For_i hardware loops with
small bodies and register-offset (dynamic) APs; matmul stationary
operands (which require static addresses) are staged into fixed SBUF
buffers with on-chip DMAs, or live at static addresses.

Precision: K/Q/values run in bf16 (the logits are divided by L/2=1024
before exp, so bf16 noise there is ~1e-6 after scaling); the two large
GEMMs (yT and the final contraction) run in float32r (~1.5e-4 rel err).
"""

import os
import sys

for _p in ("/opt/trn_rl_repo", "/root/.axon_site/_ro/trn_rl_repo"):
    if os.path.isdir(_p) and _p not in sys.path:
        sys.path.insert(0, _p)

import numpy as np
from contextlib import ExitStack

from concourse import bass, bacc, mybir, tile
from concourse.bass_utils import run_bass_kernel_spmd

P = 128
B, C, L, A = 8, 1024, 2048, 128
NC_TILES = C // P          # 8 c-tiles
NL_TILES = L // P          # 16 l-tiles
ND_TILES = C // P          # 8 d-tiles
NCHUNK = 512
NMCH = L // NCHUNK         # 4 m-chunks

F32 = mybir.dt.float32
F32R = mybir.dt.float32r
BF16 = mybir.dt.bfloat16
AF = mybir.ActivationFunctionType
ds = bass.ds

XWP_COLS = NC_TILES * L + NC_TILES * C          # x then wpT, per partition
AUX_COLS = 2 * NC_TILES * A + 2 + ND_TILES

_NC_CACHE = {}


def build_nc(rep: int = 1):
    SR = os.environ.get('KERNEL_SR', '1') == '1'
    PH = os.environ.get('BENCH_PHASES', '123')
    nc = bacc.Bacc(None, target_bir_lowering=False)

    # blob1: x [128, 8, 2048] ++ wpT [128, 8, 1024]  (f32r)
    xwp_d = nc.declare_dram_parameter("xwp", [P, XWP_COLS], F32R, isOutput=False)
    # blob2: wkT [128, 8, 128] ++ wqT [128, 8, 128] ++ bk ++ bq ++ bp [128, 8]
    aux_d = nc.declare_dram_parameter("aux", [P, AUX_COLS], F32R, isOutput=False)
    out_d = nc.declare_dram_parameter("out", [C, L], F32, isOutput=True)

    with tile.TileContext(nc) as tc, ExitStack() as octx:
        dram = octx.enter_context(tc.tile_pool(name="dram", bufs=1, space="DRAM"))
        a_dram = dram.tile([NL_TILES, P, L], F32R)

        sml = octx.enter_context(tc.tile_pool(name="sml", bufs=1))
        s_all = sml.tile([P, NL_TILES], F32)
        rs_all = sml.tile([P, NL_TILES], F32)
        bp_sb = sml.tile([P, ND_TILES], F32)

        ytp = tc.alloc_tile_pool(name="ytp", bufs=1)
        yt_sb = ytp.tile([P, NL_TILES * C], F32R)

        wkq = tc.alloc_tile_pool(name="wkq", bufs=1)
        aux_sb = wkq.tile([P, AUX_COLS], F32R)
        xwp = tc.alloc_tile_pool(name="xwp", bufs=1)
        xwp_sb = xwp.tile([P, XWP_COLS], F32R)
        kqp = tc.alloc_tile_pool(name="kqp", bufs=1)
        keys_sb = kqp.tile([P, L], BF16)
        quer_sb = kqp.tile([P, L], BF16)

        nc.sync.dma_start(out=aux_sb[:], in_=aux_d[:])
        nc.sync.dma_start(out=xwp_sb[:], in_=xwp_d[:])
        nc.vector.tensor_copy(out=bp_sb[:],
                              in_=aux_sb[:, 2 * NC_TILES * A + 2:].bitcast(F32))

        # static views
        def x_view(c):          # [128, 2048] f32r, c-tile of x
            return xwp_sb[:, c * L:(c + 1) * L]

        def wp_view(c):         # [128, 1024] f32r
            off = NC_TILES * L
            return xwp_sb[:, off + c * C:off + (c + 1) * C]

        def wk_view(c):
            return aux_sb[:, c * A:(c + 1) * A]

        def wq_view(c):
            off = NC_TILES * A
            return aux_sb[:, off + c * A:off + (c + 1) * A]

        bk_ap = aux_sb[:, 2 * NC_TILES * A:2 * NC_TILES * A + 1].bitcast(F32)
        bq_ap = aux_sb[:, 2 * NC_TILES * A + 1:2 * NC_TILES * A + 2].bitcast(F32)

        rep_ctx = tc.For_i(0, rep, 1) if rep > 1 else None
        if rep_ctx is not None:
            rep_ctx.__enter__()

        # ============ L1: keys/queries projections (4 iters) ============
        ps1 = tc.alloc_tile_pool(name="ps1", bufs=2, space="PSUM")
        if "1" in PH:
          with tc.For_i(0, NMCH, 1, staggered_reset=SR) as iv:
            for w_view, b_ap, o_sb in ((wk_view, bk_ap, keys_sb),
                                       (wq_view, bq_ap, quer_sb)):
                acc = ps1.tile([P, NCHUNK], F32, tag="ps1",
                               name="accK" if o_sb is keys_sb else "accQ")
                for c in range(NC_TILES):
                    nc.tensor.matmul(out=acc[:], lhsT=w_view(c),
                                     rhs=x_view(c)[:, ds(iv * NCHUNK, NCHUNK)],
                                     start=(c == 0), stop=(c == NC_TILES - 1))
                nc.scalar.activation(o_sb[:, ds(iv * NCHUNK, NCHUNK)], acc[:],
                                     AF.Identity, bias=b_ap)
        ps1.release()

        # ==== L23: values + exp + rowsum + yT, merged (16 iters) ====
        # rs[l-tile] depends only on this iteration's values row-block, so
        # the softmax denominator folds into the yT eviction in-iteration.
        st2 = tc.alloc_tile_pool(name="st2", bufs=1)
        k_stage = st2.tile([P, P], BF16)
        e_stage = st2.tile([P, L], F32R)
        xl_stage = st2.tile([P, NC_TILES, P], F32R)
        s_stage = st2.tile([P, 1], F32)
        rs_stage = st2.tile([P, 1], F32)
        ps23 = tc.alloc_tile_pool(name="ps23", bufs=2, space="PSUM")
        if "2" in PH:
          with tc.For_i(0, NL_TILES, 1, staggered_reset=SR) as iv:
            nc.sync.dma_start(out=k_stage[:], in_=keys_sb[:, ds(iv * P, P)])
            nc.sync.dma_start(
                out=xl_stage[:],
                in_=xwp_sb[:, :NC_TILES * L]
                    .rearrange("p (n l) -> p n l", n=NC_TILES)[:, :, ds(iv * P, P)])
            vps = ps23.tile([P, L], F32, tag="ps23")
            for j in range(NMCH):
                nc.tensor.matmul(out=vps[:, j * NCHUNK:(j + 1) * NCHUNK],
                                 lhsT=k_stage[:],
                                 rhs=quer_sb[:, j * NCHUNK:(j + 1) * NCHUNK],
                                 start=True, stop=True)
            nc.scalar.activation(e_stage[:], vps[:], AF.Exp, scale=2.0 / L,
                                 accum_out=s_stage[:])
            nc.vector.reciprocal(out=rs_stage[:], in_=s_stage[:])
            nc.sync.dma_start(
                out=a_dram.rearrange("l p m -> p l m")[:, ds(iv, 1), :],
                in_=e_stage[:])
            acc3 = ps23.tile([P, C], F32, tag="ps23", name="acc3")
            for dc in range(C // NCHUNK):
                for c in range(NC_TILES):
                    nc.tensor.matmul(
                        out=acc3[:, dc * NCHUNK:(dc + 1) * NCHUNK],
                        lhsT=xl_stage[:, c, :],
                        rhs=wp_view(c)[:, dc * NCHUNK:(dc + 1) * NCHUNK],
                        start=(c == 0), stop=(c == NC_TILES - 1))
            nc.scalar.activation(
                yt_sb[:, ds(iv * C, C)], acc3[:],
                AF.Copy, scale=rs_stage[:])
        ps23.release()
        st2.release()
        kqp.release()
        xwp.release()
        wkq.release()

        # ============ L4: out = yTs^T @ E + bp (4 iters) ============
        st4 = tc.alloc_tile_pool(name="st4", bufs=1)
        a_stage = st4.tile([P, NL_TILES, NCHUNK], F32R)
        outp = tc.alloc_tile_pool(name="outp", bufs=2)
        ps4 = tc.alloc_tile_pool(name="ps4", bufs=1, space="PSUM")
        out_v = out_d.rearrange("(n p) l -> p n l", p=P)
        if "3" in PH:
          with tc.For_i(0, NMCH, 1, staggered_reset=SR) as iv:
            a_view = a_dram.rearrange("l p m -> p l m")
            for q in range(4):
                nc.sync.dma_start(
                    out=a_stage[:, q * 4:(q + 1) * 4, :],
                    in_=a_view[:, q * 4:(q + 1) * 4, ds(iv * NCHUNK, NCHUNK)])
            accs = [ps4.tile([P, NCHUNK], F32, tag=f"ps4_{d}", name=f"acc4_{d}")
                    for d in range(ND_TILES)]
            for l in range(NL_TILES):
                for d in range(ND_TILES):
                    nc.tensor.matmul(
                        out=accs[d][:],
                        lhsT=yt_sb[:, l * C + d * P:l * C + (d + 1) * P],
                        rhs=a_stage[:, l, :],
                        start=(l == 0), stop=(l == NL_TILES - 1))
            for d in range(ND_TILES):
                o_sb = outp.tile([P, NCHUNK], F32, tag="o", name=f"o_{d % 2}")
                nc.vector.tensor_scalar_add(out=o_sb[:], in0=accs[d][:],
                                            scalar1=bp_sb[:, d:d + 1])
                nc.sync.dma_start(out=out_v[:, d, ds(iv * NCHUNK, NCHUNK)],
                                  in_=o_sb[:])
        ps4.release()
        outp.release()
        st4.release()

        if rep_ctx is not None:
            rep_ctx.__exit__(None, None, None)
        ytp.release()

    nc.compile()
    return nc


def _get_nc(rep: int = 1):
    if rep not in _NC_CACHE:
        _NC_CACHE[rep] = build_nc(rep)
    return _NC_CACHE[rep]


def make_in_maps(x, Wk, bk, Wq, bq, Wp, bp):
    x = np.asarray(x, dtype=np.float32)
    wpT = np.ascontiguousarray(np.asarray(Wp, np.float32).T)      # [C, C]
    wp_part = wpT.reshape(NC_TILES, P, C).transpose(1, 0, 2).reshape(P, NC_TILES * C)
    wkT = np.ascontiguousarray(np.asarray(Wk, np.float32).T)      # [C, A]
    wqT = np.ascontiguousarray(np.asarray(Wq, np.float32).T)
    wk_part = wkT.reshape(NC_TILES, P, A).transpose(1, 0, 2).reshape(P, NC_TILES * A)
    wq_part = wqT.reshape(NC_TILES, P, A).transpose(1, 0, 2).reshape(P, NC_TILES * A)
    aux = np.concatenate([
        wk_part, wq_part,
        np.asarray(bk, np.float32).reshape(P, 1),
        np.asarray(bq, np.float32).reshape(P, 1),
        np.ascontiguousarray(np.asarray(bp, np.float32).reshape(ND_TILES, P).T),
    ], axis=1)
    in_maps = []
    for b in range(B):
        x_part = (x[b].reshape(NC_TILES, P, L).transpose(1, 0, 2)
                  .reshape(P, NC_TILES * L))
        xwp_blob = np.concatenate([x_part, wp_part], axis=1)
        in_maps.append({"xwp": np.ascontiguousarray(xwp_blob), "aux": aux})
    return in_maps


def kernel(x, Wk, bk, Wq, bq, Wp, bp):
    nc = _get_nc(1)
    in_maps = make_in_maps(x, Wk, bk, Wq, bq, Wp, bp)
    res = run_bass_kernel_spmd(nc, in_maps, list(range(B)))
    return np.stack([res.results[b]["out"] for b in range(B)]).astype(np.float32)

